# revision 4
# baseline (speedup 1.0000x reference)
"""BlockSparseDilatedAttention TRN2 kernel.

Full inputs q,k,v: [1, 8192, 12, 64] fp32. Output: same shape.

Math: 16 blocks of 512 tokens; block pairs (r, c) with |r-c| <= 2 (74 pairs).
Per pair, dilated segment attention in 3 head-groups of 4 heads:
  g0: seg 128, dil 1 -> 4 units of 128 tokens per block
  g1: seg 256, dil 2 -> 2 units of 128 (odd positions)
  g2: seg 128, dil 4 -> 1 unit of 128 (pos 2 mod 4), block-diag mask of 4x32
Each unit: softmax(Q K^T / 8) V over its own 128 kv tokens; pair outputs are
scatter-added into the query row block.

Sharding: 8 cores = 2 head-halves x 4 row-quarters. Identical SPMD program;
edge cores get zero-padded kv blocks (zero V => zero contribution).

Device pipeline per (group, head, col-block):
  S^T = matmul(K^T stationary, Q^T moving)   [k on partitions, q free]
  A^T = exp(S^T / 8)  (ScalarE, batched across units)
  O|sums = matmul(A^T stationary, [V | 1] moving)  -> natural [q, 65] + sums col
  recip = reciprocal_approx_fast(sums)  (batched, packed on partitions)
  acc = O * recip + acc  (affine_then_add, per col-block delta)
The g2 mask is folded into 5 extra contraction rows (+-512 rank-5 term that
cancels exactly for same-subsegment pairs, and sends cross-subsegment scores
to -512 => exp ~ 0).
"""

import sys

import numpy as np

for _p in ("/opt/trn_rl_repo",):
    if _p not in sys.path:
        sys.path.append(_p)

# ---------------------------------------------------------------- constants
B, S, H, D = 1, 8192, 12, 64
BLOCK = 512
NB = S // BLOCK            # 16
GL = [512, 256, 128]       # gathered tokens per block, per group
GNU = [4, 2, 1]            # 128-token units per block, per group
GC = [64, 64, 69]          # contraction rows (g2 has 5 mask-aug rows)
MASK_M = 512.0
NCORES = 8
ROWS_PER_CORE = 4          # row blocks per quarter
CB = 8                     # col blocks per core (4 rows window, padded)
SCALE = 0.125              # 1/sqrt(64)


def _gather_pos():
    pos = [np.arange(512)]
    pos.append(np.concatenate([s + 1 + 2 * np.arange(128) for s in (0, 256)]))
    pos.append(np.concatenate([s + 2 + 4 * np.arange(32) for s in (0, 128, 256, 384)]))
    return pos


POS = _gather_pos()


def _rows_for_cb(cb):
    """Local row indices i in [0,3] attending col block cb (c = 4R-2+cb)."""
    return max(0, cb - 4), min(3, cb)


# ---------------------------------------------------------------- bass build
_BASS_CACHE = {}


def _build_bass():
    if "nc" in _BASS_CACHE:
        return _BASS_CACHE["nc"]

    import concourse.tile as tile
    from concourse import bacc, mybir

    exp_fn = mybir.ActivationFunctionType.Exp
    f32 = mybir.dt.float32
    nc = bacc.Bacc("TRN2", target_bir_lowering=False, debug=False,
                   enable_asserts=False)

    qt_d, kt_d, v_d, out_d = [], [], [], []
    for g in range(3):
        L, nu, C = GL[g], GNU[g], GC[g]
        qp = 128 if g < 2 else C       # heads stacked on partitions for g0/g1
        qf = 4 * L if g < 2 else 2 * 4 * L
        kf = CB * L if g < 2 else 2 * CB * L
        qt_d.append(nc.dram_tensor(f"qt{g}", [qp, qf], f32, kind="ExternalInput").ap())
        kt_d.append(nc.dram_tensor(f"kt{g}", [qp, kf], f32, kind="ExternalInput").ap())
        v_d.append(nc.dram_tensor(f"v{g}", [128, 2 * CB * nu * 66], f32,
                                  kind="ExternalInput").ap())
        out_d.append(nc.dram_tensor(f"out{g}", [128, 2 * 4 * nu * 64], f32,
                                    kind="ExternalOutput").ap())

    with tile.TileContext(nc) as tc:
        with tc.tile_pool(name="inp", bufs=1) as inp, \
             tc.tile_pool(name="at", bufs=6) as atp, \
             tc.tile_pool(name="small", bufs=8) as small, \
             tc.tile_pool(name="ps_s", bufs=1, space="PSUM") as ps_s, \
             tc.tile_pool(name="ps_o", bufs=3, space="PSUM") as ps_o:

            qt_sb, kt_sb, v_sb = [], [], []
            for g in range(3):
                qs = inp.tile(list(qt_d[g].shape), f32, tag=f"qt{g}")
                ks = inp.tile(list(kt_d[g].shape), f32, tag=f"kt{g}")
                vs = inp.tile(list(v_d[g].shape), f32, tag=f"v{g}")
                nc.sync.dma_start(out=qs, in_=qt_d[g])
                nc.sync.dma_start(out=ks, in_=kt_d[g])
                nc.sync.dma_start(out=vs, in_=v_d[g])
                qt_sb.append(qs)
                kt_sb.append(ks)
                v_sb.append(vs)

            zeros = inp.tile([128, 64], f32, tag="zeros")
            nc.gpsimd.memset(zeros, 0.0)

            for g in range(3):
                L, nu, C = GL[g], GNU[g], GC[g]
                if g < 2:
                    qt_r = qt_sb[g].rearrange("p (i l) -> p i l", i=4)
                    kt_r = kt_sb[g].rearrange("p (c l) -> p c l", c=CB)
                else:
                    qt_r = qt_sb[g].rearrange("p (h i l) -> p h i l", h=2, i=4)
                    kt_r = kt_sb[g].rearrange("p (h c l) -> p h c l", h=2, c=CB)
                v_r = v_sb[g].rearrange("p (h c u x) -> p h c u x", h=2, c=CB, u=nu)

                for h in range(2):
                    at_tiles = {}
                    for cb in range(CB):
                        i0, i1 = _rows_for_cb(cb)
                        nr = i1 - i0 + 1
                        # ---- scores S^T for all units of this col block
                        sp = ps_s.tile([128, nu, 512], f32, tag="s")
                        for u in range(nu):
                            if g < 2:
                                lhsT = kt_r[64 * h:64 * h + 64, cb,
                                            u * 128:(u + 1) * 128]
                                rhs = qt_r[64 * h:64 * h + 64, i0:i1 + 1,
                                           u * 128:(u + 1) * 128]
                            else:
                                lhsT = kt_r[0:C, h, cb, u * 128:(u + 1) * 128]
                                rhs = qt_r[0:C, h, i0:i1 + 1, u * 128:(u + 1) * 128]
                            nc.tensor.matmul(sp[:, u, 0:nr * 128], lhsT, rhs,
                                             start=True, stop=True)
                        at = atp.tile([128, nu, 512], f32, tag="at")
                        nc.scalar.activation(at[:, :, 0:nr * 128],
                                             sp[:, :, 0:nr * 128],
                                             exp_fn, scale=SCALE)
                        at_tiles[cb] = at

                        # ---- rows whose 5-block window just completed
                        if cb < 4:
                            continue
                        i = cb - 4
                        for u in range(nu):
                            po = ps_o.tile([128, 5, 66], f32, tag="o")
                            for d in range(5):
                                ccb = i + d
                                ci0, _ = _rows_for_cb(ccb)
                                a_t = at_tiles[ccb]
                                lhsT = a_t[:, u, (i - ci0) * 128:(i - ci0 + 1) * 128]
                                rhs = v_r[:, h, ccb, u, 0:65]
                                nc.tensor.matmul(po[:, d, 0:65], lhsT, rhs,
                                                 start=True, stop=True)
                            rc = small.tile([128, 8], f32, tag="rc")
                            nc.vector.reciprocal_approx_fast(out=rc[:, 0:5],
                                                             in_=po[:, :, 64])
                            acc = small.tile([128, 64], f32, tag="acc")
                            for d in range(5):
                                nc.vector.affine_then_add(
                                    out=acc, in0=po[:, d, 0:64],
                                    in1=zeros if d == 0 else acc,
                                    scale=rc[:, d:d + 1], bias=0.0)
                            off = ((h * 4 + i) * nu + u) * 64
                            nc.sync.dma_start(out=out_d[g][:, off:off + 64],
                                              in_=acc)

    nc.compile()
    _BASS_CACHE["nc"] = nc
    return nc


def _build_bass_cached():
    return _build_bass()


# ---------------------------------------------------------------- host pack
def _pack_inputs(q, k, v):
    """q,k,v: [1, 8192, 12, 64] fp32 -> list of 8 per-core input dicts."""
    q = np.asarray(q, dtype=np.float32)
    k = np.asarray(k, dtype=np.float32)
    v = np.asarray(v, dtype=np.float32)
    qb = q.reshape(NB, BLOCK, H, D)
    kb = k.reshape(NB, BLOCK, H, D)
    vb = v.reshape(NB, BLOCK, H, D)

    # mask-augmentation constants for g2 (gathered order: 4 subsegs of 32)
    sub = np.repeat(np.arange(4), 32)                      # [128]
    U = (sub[None, :] == np.arange(4)[:, None]).astype(np.float32)  # [4,128]

    in_maps = []
    for core in range(NCORES):
        hh, R = core // 4, core % 4
        m = {}
        for g in range(3):
            L, nu, C = GL[g], GNU[g], GC[g]
            pos = POS[g]
            if g < 2:
                qt = np.zeros((128, 4 * L), np.float32)
                kt = np.zeros((128, CB * L), np.float32)
            else:
                qt = np.zeros((C, 2 * 4 * L), np.float32)
                kt = np.zeros((C, 2 * CB * L), np.float32)
            va = np.zeros((128, 2 * CB * nu * 66), np.float32)
            for h in range(2):
                head = 4 * g + 2 * hh + h
                for i in range(ROWS_PER_CORE):
                    r = 4 * R + i
                    tok = qb[r, pos, head, :]              # [L, 64]
                    if g < 2:
                        qt[64 * h:64 * h + 64, i * L:(i + 1) * L] = tok.T
                    else:
                        o = (h * 4 + i) * L
                        qt[0:64, o:o + L] = tok.T
                        qt[64, o:o + L] = -MASK_M
                        qt[65:69, o:o + L] = MASK_M * U
                for cb in range(CB):
                    c = 4 * R - 2 + cb
                    if 0 <= c < NB:
                        tok = kb[c, pos, head, :]          # [L, 64]
                        vt = vb[c, pos, head, :]           # [L, 64]
                    else:
                        tok = np.zeros((L, D), np.float32)
                        vt = np.zeros((L, D), np.float32)
                    if g < 2:
                        kt[64 * h:64 * h + 64, cb * L:(cb + 1) * L] = tok.T
                    else:
                        o = (h * CB + cb) * L
                        kt[0:64, o:o + L] = tok.T
                        kt[64, o:o + L] = 1.0
                        kt[65:69, o:o + L] = U
                    for u in range(nu):
                        o = ((h * CB + cb) * nu + u) * 66
                        va[:, o:o + 64] = vt[u * 128:(u + 1) * 128, :]
                        va[:, o + 64] = 1.0
            m[f"qt{g}"] = qt
            m[f"kt{g}"] = kt
            m[f"v{g}"] = va
        in_maps.append(m)
    return in_maps


def _unpack(results):
    out = np.zeros((B, S, H, D), np.float32)
    for core in range(NCORES):
        hh, R = core // 4, core % 4
        res = results[core]
        for g in range(3):
            L, nu = GL[g], GNU[g]
            pos = POS[g]
            og = res[f"out{g}"]
            for h in range(2):
                head = 4 * g + 2 * hh + h
                for i in range(ROWS_PER_CORE):
                    r = 4 * R + i
                    for u in range(nu):
                        off = ((h * 4 + i) * nu + u) * 64
                        out[0, r * 512 + pos[u * 128:(u + 1) * 128], head, :] = \
                            og[:, off:off + 64]
    return out


# ---------------------------------------------------------------- entry
def _run(q, k, v, trace=False):
    from concourse.bass_utils import run_bass_kernel_spmd
    nc = _build_bass_cached()
    in_maps = _pack_inputs(q, k, v)
    res = run_bass_kernel_spmd(nc, in_maps, core_ids=list(range(NCORES)),
                               trace=trace)
    return _unpack(res.results), res


def kernel(q, k, v):
    out, _ = _run(q, k, v, trace=False)
    return out


# revision 7
# speedup vs baseline: 1.9736x; 1.9736x over previous
"""BlockSparseDilatedAttention TRN2 kernel.

Full inputs q,k,v: [1, 8192, 12, 64] fp32. Output: same shape.

Math: 16 blocks of 512 tokens; block pairs (r, c) with |r-c| <= 2 (74 pairs).
Per pair, dilated segment attention in 3 head-groups of 4 heads:
  g0: seg 128, dil 1 -> 4 units of 128 tokens per block
  g1: seg 256, dil 2 -> 2 units of 128 (odd positions)
  g2: seg 128, dil 4 -> 1 unit of 128 (pos 2 mod 4), block-diag mask of 4x32
Each unit: softmax(Q K^T / 8) V over its own 128 kv tokens; pair outputs are
scatter-added into the query row block.

Sharding: 8 cores = 2 head-halves x 4 row-quarters. Identical SPMD program;
edge cores get zero-padded kv blocks (zero V => zero contribution).

Device pipeline per (group, head, col-block):
  S^T = matmul(K^T stationary, Q^T moving)   [k on partitions, q free]
  A^T = exp(S^T / 8)  (ScalarE, batched across units)
  O|sums = matmul(A^T stationary, [V | 1] moving)  -> natural [q, 65] + sums col
  recip = reciprocal_approx_fast(sums)  (batched, packed on partitions)
  acc = O * recip + acc  (affine_then_add, per col-block delta)
The g2 mask is folded into 5 extra contraction rows (+-512 rank-5 term that
cancels exactly for same-subsegment pairs, and sends cross-subsegment scores
to -512 => exp ~ 0).
"""

import sys

import numpy as np

for _p in ("/opt/trn_rl_repo",):
    if _p not in sys.path:
        sys.path.append(_p)

# ---------------------------------------------------------------- constants
B, S, H, D = 1, 8192, 12, 64
BLOCK = 512
NB = S // BLOCK            # 16
GL = [512, 256, 128]       # gathered tokens per block, per group
GNU = [4, 2, 1]            # 128-token units per block, per group
GC = [64, 64, 69]          # contraction rows (g2 has 5 mask-aug rows)
MASK_M = 512.0
NCORES = 8
ROWS_PER_CORE = 4          # row blocks per quarter
CB = 8                     # col blocks per core (4 rows window, padded)
SCALE = 0.125              # 1/sqrt(64)


def _gather_pos():
    pos = [np.arange(512)]
    pos.append(np.concatenate([s + 1 + 2 * np.arange(128) for s in (0, 256)]))
    pos.append(np.concatenate([s + 2 + 4 * np.arange(32) for s in (0, 128, 256, 384)]))
    return pos


POS = _gather_pos()


def _rows_for_cb(cb):
    """Local row indices i in [0,3] attending col block cb (c = 4R-2+cb)."""
    return max(0, cb - 4), min(3, cb)


# ---------------------------------------------------------------- bass build
_BASS_CACHE = {}


def _build_bass():
    if "nc" in _BASS_CACHE:
        return _BASS_CACHE["nc"]

    import concourse.tile as tile
    from concourse import bacc, mybir

    exp_fn = mybir.ActivationFunctionType.Exp
    f32 = mybir.dt.float32
    nc = bacc.Bacc("TRN2", target_bir_lowering=False, debug=False,
                   enable_asserts=False)

    qt_d, kt_d, v_d, out_d = [], [], [], []
    for g in range(3):
        L, nu, C = GL[g], GNU[g], GC[g]
        qp = 128 if g < 2 else C       # heads stacked on partitions for g0/g1
        qf = 4 * L if g < 2 else 2 * 4 * L
        kf = CB * L if g < 2 else 2 * CB * L
        qt_d.append(nc.dram_tensor(f"qt{g}", [qp, qf], f32, kind="ExternalInput").ap())
        kt_d.append(nc.dram_tensor(f"kt{g}", [qp, kf], f32, kind="ExternalInput").ap())
        v_d.append(nc.dram_tensor(f"v{g}", [128, 2 * CB * nu * 66], f32,
                                  kind="ExternalInput").ap())
        out_d.append(nc.dram_tensor(f"out{g}", [128, 2 * 4 * nu * 64], f32,
                                    kind="ExternalOutput").ap())

    with tile.TileContext(nc) as tc:
        with tc.tile_pool(name="inp", bufs=1) as inp, \
             tc.tile_pool(name="at", bufs=24) as atp, \
             tc.tile_pool(name="small", bufs=8) as small, \
             tc.tile_pool(name="ps_s", bufs=2, space="PSUM") as ps_s, \
             tc.tile_pool(name="ps_o", bufs=3, space="PSUM") as ps_o:

            qt_sb, kt_sb, v_sb = [], [], []
            for g in range(3):
                qs = inp.tile(list(qt_d[g].shape), f32, tag=f"qt{g}")
                ks = inp.tile(list(kt_d[g].shape), f32, tag=f"kt{g}")
                vs = inp.tile(list(v_d[g].shape), f32, tag=f"v{g}")
                # chunked loads spread across DMA queues
                for (sb_t, dr) in ((qs, qt_d[g]), (ks, kt_d[g]), (vs, v_d[g])):
                    n = dr.shape[1]
                    nchunk = 4 if n >= 2048 else 2
                    step = n // nchunk
                    for ci in range(nchunk):
                        sl = slice(ci * step, (ci + 1) * step if ci < nchunk - 1 else n)
                        nc.sync.dma_start(out=sb_t[:, sl], in_=dr[:, sl])
                qt_sb.append(qs)
                kt_sb.append(ks)
                v_sb.append(vs)

            zeros = inp.tile([128, 64], f32, tag="zeros")
            nc.gpsimd.memset(zeros, 0.0)

            for g in range(3):
                L, nu, C = GL[g], GNU[g], GC[g]
                if g < 2:
                    qt_r = qt_sb[g].rearrange("p (i l) -> p i l", i=4)
                    kt_r = kt_sb[g].rearrange("p (c l) -> p c l", c=CB)
                else:
                    qt_r = qt_sb[g].rearrange("p (h i l) -> p h i l", h=2, i=4)
                    kt_r = kt_sb[g].rearrange("p (h c l) -> p h c l", h=2, c=CB)
                v_r = v_sb[g].rearrange("p (h c u x) -> p h c u x", h=2, c=CB, u=nu)

                at_tiles = {}
                for cb in range(CB):
                    i0, i1 = _rows_for_cb(cb)
                    nr = i1 - i0 + 1
                    # ---- scores S^T: both heads per (cb, u); the two heads'
                    # matmuls hit disjoint PE row-groups and run concurrently
                    for u in range(nu):
                        sp = ps_s.tile([128, 2, 512], f32, tag="s")
                        for h in range(2):
                            if g < 2:
                                lhsT = kt_r[64 * h:64 * h + 64, cb,
                                            u * 128:(u + 1) * 128]
                                rhs = qt_r[64 * h:64 * h + 64, i0:i1 + 1,
                                           u * 128:(u + 1) * 128]
                            else:
                                lhsT = kt_r[0:C, h, cb, u * 128:(u + 1) * 128]
                                rhs = qt_r[0:C, h, i0:i1 + 1, u * 128:(u + 1) * 128]
                            nc.tensor.matmul(sp[:, h, 0:nr * 128], lhsT, rhs,
                                             start=True, stop=True)
                        at = atp.tile([128, 2, 512], f32, tag="at")
                        nc.scalar.activation(at[:, :, 0:nr * 128],
                                             sp[:, :, 0:nr * 128],
                                             exp_fn, scale=SCALE)
                        at_tiles[(cb, u)] = at

                    # ---- rows whose 5-block window just completed
                    if cb < 4:
                        continue
                    i = cb - 4
                    for u in range(nu):
                        for h in range(2):
                            po = ps_o.tile([128, 5, 66], f32, tag="o")
                            for d in range(5):
                                ccb = i + d
                                ci0, _ = _rows_for_cb(ccb)
                                a_t = at_tiles[(ccb, u)]
                                lhsT = a_t[:, h, (i - ci0) * 128:(i - ci0 + 1) * 128]
                                rhs = v_r[:, h, ccb, u, 0:65]
                                nc.tensor.matmul(po[:, d, 0:65], lhsT, rhs,
                                                 start=True, stop=True)
                            rc = small.tile([128, 8], f32, tag="rc")
                            nc.vector.reciprocal_approx_fast(out=rc[:, 0:5],
                                                             in_=po[:, :, 64])
                            acc = small.tile([128, 64], f32, tag="acc")
                            for d in range(5):
                                nc.vector.affine_then_add(
                                    out=acc, in0=po[:, d, 0:64],
                                    in1=zeros if d == 0 else acc,
                                    scale=rc[:, d:d + 1], bias=0.0)
                            off = ((h * 4 + i) * nu + u) * 64
                            nc.sync.dma_start(out=out_d[g][:, off:off + 64],
                                              in_=acc)

    nc.compile()
    _BASS_CACHE["nc"] = nc
    return nc


def _build_bass_cached():
    return _build_bass()


# ---------------------------------------------------------------- host pack
def _pack_inputs(q, k, v):
    """q,k,v: [1, 8192, 12, 64] fp32 -> list of 8 per-core input dicts."""
    q = np.asarray(q, dtype=np.float32)
    k = np.asarray(k, dtype=np.float32)
    v = np.asarray(v, dtype=np.float32)
    qb = q.reshape(NB, BLOCK, H, D)
    kb = k.reshape(NB, BLOCK, H, D)
    vb = v.reshape(NB, BLOCK, H, D)

    # mask-augmentation constants for g2 (gathered order: 4 subsegs of 32)
    sub = np.repeat(np.arange(4), 32)                      # [128]
    U = (sub[None, :] == np.arange(4)[:, None]).astype(np.float32)  # [4,128]

    in_maps = []
    for core in range(NCORES):
        hh, R = core // 4, core % 4
        m = {}
        for g in range(3):
            L, nu, C = GL[g], GNU[g], GC[g]
            pos = POS[g]
            if g < 2:
                qt = np.zeros((128, 4 * L), np.float32)
                kt = np.zeros((128, CB * L), np.float32)
            else:
                qt = np.zeros((C, 2 * 4 * L), np.float32)
                kt = np.zeros((C, 2 * CB * L), np.float32)
            va = np.zeros((128, 2 * CB * nu * 66), np.float32)
            for h in range(2):
                head = 4 * g + 2 * hh + h
                for i in range(ROWS_PER_CORE):
                    r = 4 * R + i
                    tok = qb[r, pos, head, :]              # [L, 64]
                    if g < 2:
                        qt[64 * h:64 * h + 64, i * L:(i + 1) * L] = tok.T
                    else:
                        o = (h * 4 + i) * L
                        qt[0:64, o:o + L] = tok.T
                        qt[64, o:o + L] = -MASK_M
                        qt[65:69, o:o + L] = MASK_M * U
                for cb in range(CB):
                    c = 4 * R - 2 + cb
                    if 0 <= c < NB:
                        tok = kb[c, pos, head, :]          # [L, 64]
                        vt = vb[c, pos, head, :]           # [L, 64]
                    else:
                        tok = np.zeros((L, D), np.float32)
                        vt = np.zeros((L, D), np.float32)
                    if g < 2:
                        kt[64 * h:64 * h + 64, cb * L:(cb + 1) * L] = tok.T
                    else:
                        o = (h * CB + cb) * L
                        kt[0:64, o:o + L] = tok.T
                        kt[64, o:o + L] = 1.0
                        kt[65:69, o:o + L] = U
                    for u in range(nu):
                        o = ((h * CB + cb) * nu + u) * 66
                        va[:, o:o + 64] = vt[u * 128:(u + 1) * 128, :]
                        va[:, o + 64] = 1.0
            m[f"qt{g}"] = qt
            m[f"kt{g}"] = kt
            m[f"v{g}"] = va
        in_maps.append(m)
    return in_maps


def _unpack(results):
    out = np.zeros((B, S, H, D), np.float32)
    for core in range(NCORES):
        hh, R = core // 4, core % 4
        res = results[core]
        for g in range(3):
            L, nu = GL[g], GNU[g]
            pos = POS[g]
            og = res[f"out{g}"]
            for h in range(2):
                head = 4 * g + 2 * hh + h
                for i in range(ROWS_PER_CORE):
                    r = 4 * R + i
                    for u in range(nu):
                        off = ((h * 4 + i) * nu + u) * 64
                        out[0, r * 512 + pos[u * 128:(u + 1) * 128], head, :] = \
                            og[:, off:off + 64]
    return out


# ---------------------------------------------------------------- entry
def _run(q, k, v, trace=False):
    from concourse.bass_utils import run_bass_kernel_spmd
    nc = _build_bass_cached()
    in_maps = _pack_inputs(q, k, v)
    res = run_bass_kernel_spmd(nc, in_maps, core_ids=list(range(NCORES)),
                               trace=trace)
    return _unpack(res.results), res


def kernel(q, k, v):
    out, _ = _run(q, k, v, trace=False)
    return out


# revision 42
# speedup vs baseline: 3.4156x; 1.7307x over previous
"""BlockSparseDilatedAttention TRN2 kernel.

Full inputs q,k,v: [1, 8192, 12, 64] fp32. Output: same shape.

Math: 16 blocks of 512 tokens; block pairs (r, c) with |r-c| <= 2 (74 pairs).
Per pair, dilated segment attention in 3 head-groups of 4 heads:
  g0: seg 128, dil 1 -> 4 units of 128 tokens per block
  g1: seg 256, dil 2 -> 2 units of 128 (odd positions)
  g2: seg 128, dil 4 -> 1 unit of 128 (pos 2 mod 4), block-diag mask of 4x32
Each unit: softmax(Q K^T / 8) V over its own 128 kv tokens; pair outputs are
scatter-added into the query row block.

Sharding: 8 cores = 2 head-halves x 4 row-quarters. Identical SPMD program;
edge cores get zero-padded kv blocks (zero V => zero contribution).

Device pipeline per (group, head, col-block):
  S^T = matmul(K^T stationary, Q^T moving)   [k on partitions, q free]
  A^T = exp(S^T / 8)  (ScalarE, batched across units)
  O|sums = matmul(A^T stationary, [V | 1] moving)  -> natural [q, 65] + sums col
  recip = reciprocal_approx_fast(sums)  (batched, packed on partitions)
  acc = O * recip + acc  (affine_then_add, per col-block delta)
The g2 mask is folded into 5 extra contraction rows (+-512 rank-5 term that
cancels exactly for same-subsegment pairs, and sends cross-subsegment scores
to -512 => exp ~ 0).
"""

import sys

import numpy as np

for _p in ("/opt/trn_rl_repo",):
    if _p not in sys.path:
        sys.path.append(_p)

# ---------------------------------------------------------------- constants
B, S, H, D = 1, 8192, 12, 64
BLOCK = 512
NB = S // BLOCK            # 16
GL = [512, 256, 128]       # gathered tokens per block, per group
GNU = [4, 2, 1]            # 128-token units per block, per group
GC = [64, 64, 69]          # contraction rows (g2 has 5 mask-aug rows)
MASK_M = 512.0
NCORES = 8
ROWS_PER_CORE = 4          # row blocks per quarter
CB = 8                     # col blocks per core (4 rows window, padded)
SCALE = 0.125              # 1/sqrt(64)

# matmul input dtypes ("f32" | "f32r" | "bf16" | "f16") for the S-stage (Q,K)
# and AV-stage (A,V). f16 runs the PE at full rate (fp32 lowers to 2 half-rate
# passes) at ~4.5e-4 max rel err; set both to "f32" for ~3.5e-6 at ~1.75x time.
S_DTYPE = "f16"
AV_DTYPE = "f16"


def _gather_pos():
    pos = [np.arange(512)]
    pos.append(np.concatenate([s + 1 + 2 * np.arange(128) for s in (0, 256)]))
    pos.append(np.concatenate([s + 2 + 4 * np.arange(32) for s in (0, 128, 256, 384)]))
    return pos


POS = _gather_pos()


def _rows_for_cb(cb):
    """Local row indices i in [0,3] attending col block cb (c = 4R-2+cb)."""
    return max(0, cb - 4), min(3, cb)


# ---------------------------------------------------------------- bass build
_BASS_CACHE = {}


def _build_bass():
    if "nc" in _BASS_CACHE:
        return _BASS_CACHE["nc"]

    import concourse.tile as tile
    from concourse import bacc, mybir

    exp_fn = mybir.ActivationFunctionType.Exp
    AXIS_X = mybir.AxisListType.X
    f32 = mybir.dt.float32
    dt_map = {"f32": f32, "f32r": mybir.dt.float32r, "bf16": mybir.dt.bfloat16,
              "f16": mybir.dt.float16}
    sdt = dt_map[S_DTYPE]
    avdt = dt_map[AV_DTYPE]
    nc = bacc.Bacc("TRN2", target_bir_lowering=False, debug=False,
                   enable_asserts=False)

    qt_d, kt_d, v_d, out_d = [], [], [], []
    for g in range(3):
        L, nu, C = GL[g], GNU[g], GC[g]
        qp = 128 if g < 2 else C       # heads stacked on partitions for g0/g1
        qf = 4 * L if g < 2 else 2 * 4 * L
        kf = CB * L if g < 2 else 2 * CB * L
        qt_d.append(nc.dram_tensor(f"qt{g}", [qp, qf], sdt, kind="ExternalInput").ap())
        kt_d.append(nc.dram_tensor(f"kt{g}", [qp, kf], sdt, kind="ExternalInput").ap())
        v_d.append(nc.dram_tensor(f"v{g}", [128, 2 * CB * nu * 66], avdt,
                                  kind="ExternalInput").ap())
        out_d.append(nc.dram_tensor(f"out{g}", [128, 2 * 4 * nu * 64], f32,
                                    kind="ExternalOutput").ap())

    with tile.TileContext(nc) as tc:
        with tc.tile_pool(name="inp", bufs=1) as inp, \
             tc.tile_pool(name="at", bufs=42) as atp, \
             tc.tile_pool(name="small", bufs=8) as small, \
             tc.tile_pool(name="ps_s", bufs=2, space="PSUM") as ps_s, \
             tc.tile_pool(name="ps_o", bufs=2, space="PSUM") as ps_o:

            qt_sb, kt_sb, v_sb = [], [], []
            loads = []      # (priority, sbuf_tile, dram_ap, fracs)
            # graded chunks: small at the need-front so compute starts early
            front = (1 / 16, 1 / 16, 1 / 8, 1 / 4, 1 / 4, 1 / 4)
            back = (1 / 4, 1 / 4, 1 / 2)
            for g in range(3):
                qs = inp.tile(list(qt_d[g].shape), sdt, tag=f"qt{g}")
                ks = inp.tile(list(kt_d[g].shape), sdt, tag=f"kt{g}")
                vs = inp.tile(list(v_d[g].shape), avdt, tag=f"v{g}")
                loads.append((g * 10 + 0, ks, kt_d[g], front if g == 0 else back))
                loads.append((g * 10 + 1, qs, qt_d[g], front if g == 0 else back))
                loads.append((g * 10 + 2, vs, v_d[g], back))
                qt_sb.append(qs)
                kt_sb.append(ks)
                v_sb.append(vs)
            iss = [nc.sync, nc.sync]
            qi = 0
            for _, sb_t, dr, fracs in sorted(loads, key=lambda t: t[0]):
                n = dr.shape[1]
                pos = 0
                for fi, fr in enumerate(fracs):
                    end = n if fi == len(fracs) - 1 else min(n, pos + max(64, int(n * fr) // 64 * 64))
                    if end <= pos:
                        continue
                    iss[qi % 2].dma_start(out=sb_t[:, pos:end], in_=dr[:, pos:end])
                    qi += 1
                    pos = end

            out_sb = []
            for g in range(3):
                osb = inp.tile(list(out_d[g].shape), f32, tag=f"out{g}")
                out_sb.append(osb)

            qt_r, kt_r, v_r = [], [], []
            for g in range(3):
                nu = GNU[g]
                if g < 2:
                    qt_r.append(qt_sb[g].rearrange("p (i l) -> p i l", i=4))
                    kt_r.append(kt_sb[g].rearrange("p (c l) -> p c l", c=CB))
                else:
                    qt_r.append(qt_sb[g].rearrange("p (h i l) -> p h i l",
                                                   h=2, i=4))
                    kt_r.append(kt_sb[g].rearrange("p (h c l) -> p h c l",
                                                   h=2, c=CB))
                v_r.append(v_sb[g].rearrange("p (h c u x) -> p h c u x",
                                             h=2, c=CB, u=nu))

            at_tiles = {}
            for g in range(3):
                for cb in range(CB):
                    i0, i1 = _rows_for_cb(cb)
                    nr = i1 - i0 + 1
                    # ---- scores S^T: both heads per (g, cb, u); the two
                    # heads' matmuls hit disjoint PE row-groups concurrently
                    nu, C = GNU[g], GC[g]
                    for u in range(nu):
                        sp = ps_s.tile([128, 2, 512], f32, tag="s")
                        for h in range(2):
                            if g < 2:
                                lhsT = kt_r[g][64 * h:64 * h + 64, cb,
                                              u * 128:(u + 1) * 128]
                                rhs = qt_r[g][64 * h:64 * h + 64, i0:i1 + 1,
                                              u * 128:(u + 1) * 128]
                            else:
                                lhsT = kt_r[g][0:C, h, cb, u * 128:(u + 1) * 128]
                                rhs = qt_r[g][0:C, h, i0:i1 + 1,
                                              u * 128:(u + 1) * 128]
                            nc.tensor.matmul(sp[:, h, 0:nr * 128], lhsT, rhs,
                                             start=True, stop=True)
                        at = atp.tile([128, 2, 512], avdt, tag="at")
                        nc.scalar.activation(at[:, :, 0:nr * 128],
                                             sp[:, :, 0:nr * 128],
                                             exp_fn, scale=SCALE)
                        at_tiles[(g, cb, u)] = at

                    # ---- rows whose 5-block window just completed
                    if cb < 4:
                        continue
                    i = cb - 4
                    upairs = [(0, 1), (2, 3)] if nu == 4 else \
                             ([(0, 1)] if nu == 2 else [(0,)])
                    for up in upairs:
                        for h in range(2):
                            nw = len(up)
                            po = ps_o.tile([128, 2, 512], f32, tag="o")
                            for d in range(5):
                                ccb = i + d
                                ci0, _ = _rows_for_cb(ccb)
                                for us, u in enumerate(up):
                                    a_t = at_tiles[(g, ccb, u)]
                                    lhsT = a_t[:, h,
                                               (i - ci0) * 128:(i - ci0 + 1) * 128]
                                    rhs = v_r[g][:, h, ccb, u, 0:65]
                                    nc.tensor.matmul(po[:, us, d * 66:d * 66 + 65],
                                                     lhsT, rhs,
                                                     start=True, stop=True)
                            pv = po[:, :, 0:462].rearrange("p u (c x) -> p u c x",
                                                           x=66)
                            rc = small.tile([128, 2, 8], f32, tag="rc")
                            nc.vector.reciprocal_approx_fast(
                                out=rc[:, 0:nw, 0:5], in_=pv[:, 0:nw, 0:5, 64])
                            # tmp[q, us, delta, d] = po * rc  (contiguous write)
                            tmp = small.tile([128, 2, 5, 64], f32, tag="tmp")
                            nc.vector.tensor_mul(
                                tmp[:, 0:nw], pv[:, 0:nw, 0:5, 0:64],
                                rc[:, 0:nw, 0:5].broadcast_to([128, nw, 5, 64]))
                            off = ((h * 4 + i) * nu + up[0]) * 64
                            dst = out_sb[g][:, off:off + nw * 64]
                            if g == 0:
                                # delta-sum as an add tree on the idle GpSimd
                                sc = small.tile([128, 2, 64], f32, tag="sc")
                                sc2 = small.tile([128, 2, 64], f32, tag="sc2")
                                nc.gpsimd.tensor_add(sc, tmp[:, :, 0, :],
                                                     tmp[:, :, 1, :])
                                nc.gpsimd.tensor_add(sc2, tmp[:, :, 2, :],
                                                     tmp[:, :, 3, :])
                                nc.gpsimd.tensor_add(sc, sc, sc2)
                                nc.gpsimd.tensor_add(
                                    dst.rearrange("p (u d) -> p u d", d=64),
                                    sc, tmp[:, :, 4, :])
                            else:
                                nc.vector.reduce_sum(
                                    dst,
                                    tmp[:, 0:nw].rearrange("p u c d -> p u d c"),
                                    axis=AXIS_X)

                    # ship finished output rows early (all u of both h, row i)
                    row = 4 * nu * 64
                    o0 = i * nu * 64
                    nc.sync.dma_start(out=out_d[g][:, o0:o0 + nu * 64],
                                      in_=out_sb[g][:, o0:o0 + nu * 64])
                    nc.sync.dma_start(out=out_d[g][:, row + o0:row + o0 + nu * 64],
                                      in_=out_sb[g][:, row + o0:row + o0 + nu * 64])

    nc.compile()
    _BASS_CACHE["nc"] = nc
    return nc


def _build_bass_cached():
    return _build_bass()


# ---------------------------------------------------------------- host pack
def _np_dtype(name):
    if name == "bf16":
        import ml_dtypes
        return ml_dtypes.bfloat16
    if name == "f16":
        return np.float16
    return np.float32


def _pack_inputs(q, k, v):
    """q,k,v: [1, 8192, 12, 64] fp32 -> list of 8 per-core input dicts."""
    q = np.asarray(q, dtype=np.float32)
    k = np.asarray(k, dtype=np.float32)
    v = np.asarray(v, dtype=np.float32)
    s_np = _np_dtype(S_DTYPE)
    av_np = _np_dtype(AV_DTYPE)
    qb = q.reshape(NB, BLOCK, H, D)
    kb = k.reshape(NB, BLOCK, H, D)
    vb = v.reshape(NB, BLOCK, H, D)

    # mask-augmentation constants for g2 (gathered order: 4 subsegs of 32)
    sub = np.repeat(np.arange(4), 32)                      # [128]
    U = (sub[None, :] == np.arange(4)[:, None]).astype(np.float32)  # [4,128]

    in_maps = []
    for core in range(NCORES):
        hh, R = core // 4, core % 4
        m = {}
        for g in range(3):
            L, nu, C = GL[g], GNU[g], GC[g]
            pos = POS[g]
            if g < 2:
                qt = np.zeros((128, 4 * L), np.float32)
                kt = np.zeros((128, CB * L), np.float32)
            else:
                qt = np.zeros((C, 2 * 4 * L), np.float32)
                kt = np.zeros((C, 2 * CB * L), np.float32)
            va = np.zeros((128, 2 * CB * nu * 66), np.float32)
            for h in range(2):
                head = 4 * g + 2 * hh + h
                for i in range(ROWS_PER_CORE):
                    r = 4 * R + i
                    tok = qb[r, pos, head, :]              # [L, 64]
                    if g < 2:
                        qt[64 * h:64 * h + 64, i * L:(i + 1) * L] = tok.T
                    else:
                        o = (h * 4 + i) * L
                        qt[0:64, o:o + L] = tok.T
                        qt[64, o:o + L] = -MASK_M
                        qt[65:69, o:o + L] = MASK_M * U
                for cb in range(CB):
                    c = 4 * R - 2 + cb
                    if 0 <= c < NB:
                        tok = kb[c, pos, head, :]          # [L, 64]
                        vt = vb[c, pos, head, :]           # [L, 64]
                    else:
                        tok = np.zeros((L, D), np.float32)
                        vt = np.zeros((L, D), np.float32)
                    if g < 2:
                        kt[64 * h:64 * h + 64, cb * L:(cb + 1) * L] = tok.T
                    else:
                        o = (h * CB + cb) * L
                        kt[0:64, o:o + L] = tok.T
                        kt[64, o:o + L] = 1.0
                        kt[65:69, o:o + L] = U
                    for u in range(nu):
                        o = ((h * CB + cb) * nu + u) * 66
                        va[:, o:o + 64] = vt[u * 128:(u + 1) * 128, :]
                        va[:, o + 64] = 1.0
            m[f"qt{g}"] = qt.astype(s_np)
            m[f"kt{g}"] = kt.astype(s_np)
            m[f"v{g}"] = va.astype(av_np)
        in_maps.append(m)
    return in_maps


def _unpack(results):
    out = np.zeros((B, S, H, D), np.float32)
    for core in range(NCORES):
        hh, R = core // 4, core % 4
        res = results[core]
        for g in range(3):
            L, nu = GL[g], GNU[g]
            pos = POS[g]
            og = res[f"out{g}"]
            for h in range(2):
                head = 4 * g + 2 * hh + h
                for i in range(ROWS_PER_CORE):
                    r = 4 * R + i
                    for u in range(nu):
                        off = ((h * 4 + i) * nu + u) * 64
                        out[0, r * 512 + pos[u * 128:(u + 1) * 128], head, :] = \
                            og[:, off:off + 64]
    return out


# ---------------------------------------------------------------- entry
def _run(q, k, v, trace=False):
    from concourse.bass_utils import run_bass_kernel_spmd
    nc = _build_bass_cached()
    in_maps = _pack_inputs(q, k, v)
    res = run_bass_kernel_spmd(nc, in_maps, core_ids=list(range(NCORES)),
                               trace=trace)
    return _unpack(res.results), res


def kernel(q, k, v):
    out, _ = _run(q, k, v, trace=False)
    return out


# revision 47
# speedup vs baseline: 3.4755x; 1.0175x over previous
"""BlockSparseDilatedAttention TRN2 kernel.

Full inputs q,k,v: [1, 8192, 12, 64] fp32. Output: same shape.

Math: 16 blocks of 512 tokens; block pairs (r, c) with |r-c| <= 2 (74 pairs).
Per pair, dilated segment attention in 3 head-groups of 4 heads:
  g0: seg 128, dil 1 -> 4 units of 128 tokens per block
  g1: seg 256, dil 2 -> 2 units of 128 (odd positions)
  g2: seg 128, dil 4 -> 1 unit of 128 (pos 2 mod 4), block-diag mask of 4x32
Each unit: softmax(Q K^T / 8) V over its own 128 kv tokens; pair outputs are
scatter-added into the query row block.

Sharding: 8 cores = 2 head-halves x 4 row-quarters. Identical SPMD program;
edge cores get zero-padded kv blocks (zero V => zero contribution).

Device pipeline per (group, head, col-block):
  S^T = matmul(K^T stationary, Q^T moving)   [k on partitions, q free]
  A^T = exp(S^T / 8)  (ScalarE, batched across units)
  O|sums = matmul(A^T stationary, [V | 1] moving)  -> natural [q, 65] + sums col
  recip = reciprocal_approx_fast(sums)  (batched, packed on partitions)
  acc = O * recip + acc  (affine_then_add, per col-block delta)
The g2 mask is folded into 5 extra contraction rows (+-512 rank-5 term that
cancels exactly for same-subsegment pairs, and sends cross-subsegment scores
to -512 => exp ~ 0).
"""

import sys

import numpy as np

for _p in ("/opt/trn_rl_repo",):
    if _p not in sys.path:
        sys.path.append(_p)

# ---------------------------------------------------------------- constants
B, S, H, D = 1, 8192, 12, 64
BLOCK = 512
NB = S // BLOCK            # 16
GL = [512, 256, 128]       # gathered tokens per block, per group
GNU = [4, 2, 1]            # 128-token units per block, per group
GC = [64, 64, 69]          # contraction rows (g2 has 5 mask-aug rows)
MASK_M = 512.0
NCORES = 8
ROWS_PER_CORE = 4          # row blocks per quarter
CB = 8                     # col blocks per core (4 rows window, padded)
SCALE = 0.125              # 1/sqrt(64)

# matmul input dtypes ("f32" | "f32r" | "bf16" | "f16") for the S-stage (Q,K)
# and AV-stage (A,V). f16 runs the PE at full rate (fp32 lowers to 2 half-rate
# passes) at ~4.5e-4 max rel err; set both to "f32" for ~3.5e-6 at ~1.75x time.
S_DTYPE = "f16"
AV_DTYPE = "f16"


def _gather_pos():
    pos = [np.arange(512)]
    pos.append(np.concatenate([s + 1 + 2 * np.arange(128) for s in (0, 256)]))
    pos.append(np.concatenate([s + 2 + 4 * np.arange(32) for s in (0, 128, 256, 384)]))
    return pos


POS = _gather_pos()


def _rows_for_cb(cb):
    """Local row indices i in [0,3] attending col block cb (c = 4R-2+cb)."""
    return max(0, cb - 4), min(3, cb)


# ---------------------------------------------------------------- bass build
_BASS_CACHE = {}


def _build_bass():
    if "nc" in _BASS_CACHE:
        return _BASS_CACHE["nc"]

    import concourse.tile as tile
    from concourse import bacc, mybir

    exp_fn = mybir.ActivationFunctionType.Exp
    AXIS_X = mybir.AxisListType.X
    f32 = mybir.dt.float32
    dt_map = {"f32": f32, "f32r": mybir.dt.float32r, "bf16": mybir.dt.bfloat16,
              "f16": mybir.dt.float16}
    sdt = dt_map[S_DTYPE]
    avdt = dt_map[AV_DTYPE]
    nc = bacc.Bacc("TRN2", target_bir_lowering=False, debug=False,
                   enable_asserts=False)

    qt_d, kt_d, v_d, out_d = [], [], [], []
    for g in range(3):
        L, nu, C = GL[g], GNU[g], GC[g]
        qp = 128 if g < 2 else C       # heads stacked on partitions for g0/g1
        qf = 4 * L if g < 2 else 2 * 4 * L
        kf = CB * L if g < 2 else 2 * CB * L
        qt_d.append(nc.dram_tensor(f"qt{g}", [qp, qf], sdt, kind="ExternalInput").ap())
        kt_d.append(nc.dram_tensor(f"kt{g}", [qp, kf], sdt, kind="ExternalInput").ap())
        v_d.append(nc.dram_tensor(f"v{g}", [128, 2 * CB * nu * 66], avdt,
                                  kind="ExternalInput").ap())
        out_d.append(nc.dram_tensor(f"out{g}", [128, 2 * 4 * nu * 64], f32,
                                    kind="ExternalOutput").ap())

    with tile.TileContext(nc) as tc:
        with tc.tile_pool(name="inp", bufs=1) as inp, \
             tc.tile_pool(name="at", bufs=42) as atp, \
             tc.tile_pool(name="small", bufs=14) as small, \
             tc.tile_pool(name="ps_s", bufs=2, space="PSUM") as ps_s, \
             tc.tile_pool(name="ps_o", bufs=2, space="PSUM") as ps_o:

            qt_sb, kt_sb, v_sb = [], [], []
            loads = []      # (priority, sbuf_tile, dram_ap, fracs)
            # graded chunks: small at the need-front so compute starts early
            front = (1 / 16, 1 / 16, 1 / 8, 1 / 4, 1 / 4, 1 / 4)
            back = (1 / 4, 1 / 4, 1 / 2)
            for g in range(3):
                qs = inp.tile(list(qt_d[g].shape), sdt, tag=f"qt{g}")
                ks = inp.tile(list(kt_d[g].shape), sdt, tag=f"kt{g}")
                vs = inp.tile(list(v_d[g].shape), avdt, tag=f"v{g}")
                loads.append((g * 10 + 0, ks, kt_d[g], front if g == 0 else back))
                loads.append((g * 10 + 1, qs, qt_d[g], front if g == 0 else back))
                loads.append((g * 10 + 2, vs, v_d[g], back))
                qt_sb.append(qs)
                kt_sb.append(ks)
                v_sb.append(vs)
            iss = [nc.sync, nc.sync]
            qi = 0
            for _, sb_t, dr, fracs in sorted(loads, key=lambda t: t[0]):
                n = dr.shape[1]
                pos = 0
                for fi, fr in enumerate(fracs):
                    end = n if fi == len(fracs) - 1 else min(n, pos + max(64, int(n * fr) // 64 * 64))
                    if end <= pos:
                        continue
                    iss[qi % 2].dma_start(out=sb_t[:, pos:end], in_=dr[:, pos:end])
                    qi += 1
                    pos = end

            out_sb = []
            for g in range(3):
                osb = inp.tile(list(out_d[g].shape), f32, tag=f"out{g}")
                out_sb.append(osb)

            qt_r, kt_r, v_r = [], [], []
            for g in range(3):
                nu = GNU[g]
                if g < 2:
                    qt_r.append(qt_sb[g].rearrange("p (i l) -> p i l", i=4))
                    kt_r.append(kt_sb[g].rearrange("p (c l) -> p c l", c=CB))
                else:
                    qt_r.append(qt_sb[g].rearrange("p (h i l) -> p h i l",
                                                   h=2, i=4))
                    kt_r.append(kt_sb[g].rearrange("p (h c l) -> p h c l",
                                                   h=2, c=CB))
                v_r.append(v_sb[g].rearrange("p (h c u x) -> p h c u x",
                                             h=2, c=CB, u=nu))

            at_tiles = {}
            for g in range(3):
                for cb in range(CB):
                    i0, i1 = _rows_for_cb(cb)
                    nr = i1 - i0 + 1
                    # ---- scores S^T: both heads per (g, cb, u); the two
                    # heads' matmuls hit disjoint PE row-groups concurrently
                    nu, C = GNU[g], GC[g]
                    for u in range(nu):
                        sp = ps_s.tile([128, 2, 512], f32, tag="s")
                        for h in range(2):
                            if g < 2:
                                lhsT = kt_r[g][64 * h:64 * h + 64, cb,
                                              u * 128:(u + 1) * 128]
                                rhs = qt_r[g][64 * h:64 * h + 64, i0:i1 + 1,
                                              u * 128:(u + 1) * 128]
                            else:
                                lhsT = kt_r[g][0:C, h, cb, u * 128:(u + 1) * 128]
                                rhs = qt_r[g][0:C, h, i0:i1 + 1,
                                              u * 128:(u + 1) * 128]
                            nc.tensor.matmul(sp[:, h, 0:nr * 128], lhsT, rhs,
                                             start=True, stop=True)
                        at = atp.tile([128, 2, 512], avdt, tag="at")
                        nc.scalar.activation(at[:, :, 0:nr * 128],
                                             sp[:, :, 0:nr * 128],
                                             exp_fn, scale=SCALE)
                        at_tiles[(g, cb, u)] = at

                    # ---- rows whose 5-block window just completed
                    if cb < 4:
                        continue
                    i = cb - 4
                    upairs = [(0, 1), (2, 3)] if nu == 4 else \
                             ([(0, 1)] if nu == 2 else [(0,)])
                    for up in upairs:
                        for h in range(2):
                            nw = len(up)
                            po = ps_o.tile([128, 2, 512], f32, tag="o")
                            for d in range(5):
                                ccb = i + d
                                ci0, _ = _rows_for_cb(ccb)
                                for us, u in enumerate(up):
                                    a_t = at_tiles[(g, ccb, u)]
                                    lhsT = a_t[:, h,
                                               (i - ci0) * 128:(i - ci0 + 1) * 128]
                                    rhs = v_r[g][:, h, ccb, u, 0:65]
                                    nc.tensor.matmul(po[:, us, d * 66:d * 66 + 65],
                                                     lhsT, rhs,
                                                     start=True, stop=True)
                            pv = po[:, :, 0:462].rearrange("p u (c x) -> p u c x",
                                                           x=66)
                            rc = small.tile([128, 2, 8], f32, tag="rc")
                            nc.vector.reciprocal_approx_fast(
                                out=rc[:, 0:nw, 0:5], in_=pv[:, 0:nw, 0:5, 64])
                            # tmp[q, us, delta, d] = po * rc  (contiguous write)
                            tmp = small.tile([128, 2, 5, 64], f32, tag="tmp")
                            nc.vector.tensor_mul(
                                tmp[:, 0:nw], pv[:, 0:nw, 0:5, 0:64],
                                rc[:, 0:nw, 0:5].broadcast_to([128, nw, 5, 64]))
                            off = ((h * 4 + i) * nu + up[0]) * 64
                            dst = out_sb[g][:, off:off + nw * 64]
                            if g == 0:
                                # delta-sum as an add tree on the idle GpSimd
                                sc = small.tile([128, 2, 64], f32, tag="sc")
                                sc2 = small.tile([128, 2, 64], f32, tag="sc2")
                                nc.gpsimd.tensor_add(sc, tmp[:, :, 0, :],
                                                     tmp[:, :, 1, :])
                                nc.gpsimd.tensor_add(sc2, tmp[:, :, 2, :],
                                                     tmp[:, :, 3, :])
                                nc.gpsimd.tensor_add(sc, sc, sc2)
                                nc.gpsimd.tensor_add(
                                    dst.rearrange("p (u d) -> p u d", d=64),
                                    sc, tmp[:, :, 4, :])
                            else:
                                nc.vector.reduce_sum(
                                    dst,
                                    tmp[:, 0:nw].rearrange("p u c d -> p u d c"),
                                    axis=AXIS_X)

                    # ship finished output rows early (all u of both h, row i)
                    row = 4 * nu * 64
                    o0 = i * nu * 64
                    nc.sync.dma_start(out=out_d[g][:, o0:o0 + nu * 64],
                                      in_=out_sb[g][:, o0:o0 + nu * 64])
                    nc.sync.dma_start(out=out_d[g][:, row + o0:row + o0 + nu * 64],
                                      in_=out_sb[g][:, row + o0:row + o0 + nu * 64])

    nc.compile()
    _BASS_CACHE["nc"] = nc
    return nc


def _build_bass_cached():
    return _build_bass()


# ---------------------------------------------------------------- host pack
def _np_dtype(name):
    if name == "bf16":
        import ml_dtypes
        return ml_dtypes.bfloat16
    if name == "f16":
        return np.float16
    return np.float32


def _pack_inputs(q, k, v):
    """q,k,v: [1, 8192, 12, 64] fp32 -> list of 8 per-core input dicts."""
    q = np.asarray(q, dtype=np.float32)
    k = np.asarray(k, dtype=np.float32)
    v = np.asarray(v, dtype=np.float32)
    s_np = _np_dtype(S_DTYPE)
    av_np = _np_dtype(AV_DTYPE)
    qb = q.reshape(NB, BLOCK, H, D)
    kb = k.reshape(NB, BLOCK, H, D)
    vb = v.reshape(NB, BLOCK, H, D)

    # mask-augmentation constants for g2 (gathered order: 4 subsegs of 32)
    sub = np.repeat(np.arange(4), 32)                      # [128]
    U = (sub[None, :] == np.arange(4)[:, None]).astype(np.float32)  # [4,128]

    in_maps = []
    for core in range(NCORES):
        hh, R = core // 4, core % 4
        m = {}
        for g in range(3):
            L, nu, C = GL[g], GNU[g], GC[g]
            pos = POS[g]
            if g < 2:
                qt = np.zeros((128, 4 * L), np.float32)
                kt = np.zeros((128, CB * L), np.float32)
            else:
                qt = np.zeros((C, 2 * 4 * L), np.float32)
                kt = np.zeros((C, 2 * CB * L), np.float32)
            va = np.zeros((128, 2 * CB * nu * 66), np.float32)
            for h in range(2):
                head = 4 * g + 2 * hh + h
                for i in range(ROWS_PER_CORE):
                    r = 4 * R + i
                    tok = qb[r, pos, head, :]              # [L, 64]
                    if g < 2:
                        qt[64 * h:64 * h + 64, i * L:(i + 1) * L] = tok.T
                    else:
                        o = (h * 4 + i) * L
                        qt[0:64, o:o + L] = tok.T
                        qt[64, o:o + L] = -MASK_M
                        qt[65:69, o:o + L] = MASK_M * U
                for cb in range(CB):
                    c = 4 * R - 2 + cb
                    if 0 <= c < NB:
                        tok = kb[c, pos, head, :]          # [L, 64]
                        vt = vb[c, pos, head, :]           # [L, 64]
                    else:
                        tok = np.zeros((L, D), np.float32)
                        vt = np.zeros((L, D), np.float32)
                    if g < 2:
                        kt[64 * h:64 * h + 64, cb * L:(cb + 1) * L] = tok.T
                    else:
                        o = (h * CB + cb) * L
                        kt[0:64, o:o + L] = tok.T
                        kt[64, o:o + L] = 1.0
                        kt[65:69, o:o + L] = U
                    for u in range(nu):
                        o = ((h * CB + cb) * nu + u) * 66
                        va[:, o:o + 64] = vt[u * 128:(u + 1) * 128, :]
                        va[:, o + 64] = 1.0
            m[f"qt{g}"] = qt.astype(s_np)
            m[f"kt{g}"] = kt.astype(s_np)
            m[f"v{g}"] = va.astype(av_np)
        in_maps.append(m)
    return in_maps


def _unpack(results):
    out = np.zeros((B, S, H, D), np.float32)
    for core in range(NCORES):
        hh, R = core // 4, core % 4
        res = results[core]
        for g in range(3):
            L, nu = GL[g], GNU[g]
            pos = POS[g]
            og = res[f"out{g}"]
            for h in range(2):
                head = 4 * g + 2 * hh + h
                for i in range(ROWS_PER_CORE):
                    r = 4 * R + i
                    for u in range(nu):
                        off = ((h * 4 + i) * nu + u) * 64
                        out[0, r * 512 + pos[u * 128:(u + 1) * 128], head, :] = \
                            og[:, off:off + 64]
    return out


# ---------------------------------------------------------------- entry
def _run(q, k, v, trace=False):
    from concourse.bass_utils import run_bass_kernel_spmd
    nc = _build_bass_cached()
    in_maps = _pack_inputs(q, k, v)
    res = run_bass_kernel_spmd(nc, in_maps, core_ids=list(range(NCORES)),
                               trace=trace)
    return _unpack(res.results), res


def kernel(q, k, v):
    out, _ = _run(q, k, v, trace=False)
    return out
